# revision 1
# baseline (speedup 1.0000x reference)
"""Trainium2 Bass kernel for nn_EnhancedFeatureLayer (OHLCV feature extraction
+ per-instance normalization over the sequence axis).

Input : x [131072, 24, 5] fp32 (open, high, low, close, volume)
Output:   [131072, 24, 25] fp32 (25 features, instance-normalized over s)

Strategy (pure data parallel over 8 NeuronCores, 16384 batches each):
  - Load batches b-on-partitions (contiguous DMA), blocks of 1024 batches.
  - All compute in a CHANNEL-MAJOR scratch ft [128, G, 21, 24] so feature
    writes, squares, stats reductions and the scale pass are contiguous
    (the s-major staging layout of v1 paid a ~2-7ns/elem stride penalty on
    nearly every access; that was the whole bottleneck).
  - One 118-row transposed pack per chunk (close|vol|gain|loss|vmult|ones)
    feeds a single [118, 240] matmul producing every linear sequence map
    (EMAs, 14-window averages, OBV cumsum, +eps biases). returns/mom3/mom6
    are shifted-AP subtractions in b-on-P (cheaper than matmul columns).
  - hour_sin/cos are batch-independent: host-normalized constants copied
    straight into the store tile. ch23/24 normalize to exactly 0.
  - Stats: contiguous square (ACT) + two contiguous DVE reduces; rsqrt via
    exp(-0.5*ln). Normalize = (f*A) contiguous in-place, then one strided
    (f + B) pass that also transposes c-major -> s-major into the store
    tile, split across DVE/GPSIMD with wide channel runs.
"""

import math
import sys

import numpy as np

for _p in ("/opt/trn_rl_repo",):
    if _p not in sys.path:
        sys.path.insert(0, _p)

EPS = 1e-8
IN_EPS = 1e-5
S = 24
CIN = 5
COUT = 25
NFT = 21          # channels that need stats+affine (out 0..15, 18..22)
PB = 128          # batches per chunk (partition dim)
G = 8             # chunks per block
BLOCK = PB * G    # 1024 batches
NCORES = 8

NPK = 118         # transposed pack rows: close 0:24 | vol 24:48 | gain 48:71
                  #   | loss 71:94 | vmult 94:117 | ones 117
NMM = 240         # matmul output columns per chunk:
#   0:96    ema3|6|12(+eps)|24 s      96:144  vol6|vol12
#   144:168 avgL(+eps)               168:192 avgS(g+l+eps)
#   192:216 vol(copy)                216:240 obv
F32R = True
ACT_SET = "natural_log_exp_and_others"


# --------------------------------------------------------------------------
# host-side constants
# --------------------------------------------------------------------------

def _ema_mat(span):
    """[24, 24]; column s holds the weights over close[0..s]."""
    a = 2.0 / (span + 1)
    pows = (1.0 - a) ** np.arange(S, dtype=np.float64)
    W = np.zeros((S, S))
    for s in range(S):
        W[: s + 1, s] = pows[: s + 1] / pows[: s + 1].sum()
    return W


def _mavg_pad_mat():
    """[23, 24]; col s = replicate-padded 14-window avg at index max(s-1, 0)."""
    M = np.zeros((23, 23))
    for l in range(23):
        M[0, l] += max(13 - l, 0) / 14.0
        for j in range(max(0, l - 13), l + 1):
            M[j, l] += 1.0 / 14.0
    P = np.zeros((23, 24))
    P[:, 0] = M[:, 0]
    for s in range(1, 24):
        P[:, s] = M[:, s - 1]
    return P


def _build_weights():
    """W [118, NMM] applied to [close(24); vol(24); gain(23); loss(23);
    vmult(23); 1]."""
    W = np.zeros((NPK, NMM))
    E = {k: _ema_mat(k) for k in (3, 6, 12, 24)}
    W[0:24, 0:24] = E[3]
    W[0:24, 24:48] = E[6]
    W[0:24, 48:72] = E[12]
    W[0:24, 72:96] = E[24]
    W[24:48, 96:120] = E[6]
    W[24:48, 120:144] = E[12]
    W[117, 48:72] = EPS              # ema12s + eps (denominator of ch20)
    P = _mavg_pad_mat()
    W[71:94, 144:168] = P            # avgL from loss
    W[117, 144:168] = EPS            # avgL + eps
    W[48:71, 168:192] = P            # avgS = g-part
    W[71:94, 168:192] += P           #        + l-part
    W[117, 168:192] = EPS            #        + eps
    W[24:48, 192:216] = np.eye(24)   # vol passthrough (for rv = exp(-ln(vol)))
    for s in range(24):              # obv from vmult (rows are s'=1..23)
        W[94 : 94 + s, 216 + s] = 1.0
    return W.astype(np.float32)


def _hour_consts():
    """Normalized hour_sin / hour_cos, channel-major [2, 24] -> flat [48]."""
    t = np.arange(S, dtype=np.float32)
    ang = (np.float32(2.0 * math.pi) * (t % 24) / np.float32(24.0)).astype(np.float32)
    out = np.empty((2, S), dtype=np.float32)
    for i, v in enumerate((np.sin(ang).astype(np.float32),
                           np.cos(ang).astype(np.float32))):
        m = v.mean(dtype=np.float32)
        var = v.var(dtype=np.float32)
        out[i, :] = (v - m) / np.sqrt(var + np.float32(IN_EPS))
    return out.reshape(-1)


# DRAM output is CHANNEL-MAJOR [b, 25ft, 24s]; ft channel -> out channel:
# ft 0..15 = out 0..15, ft 16..20 = out 18..22, ft 21/22 = hour sin/cos
# (out 16/17), ft 23/24 = out 23/24 (exact zeros). The host gathers with
# PERM and transposes (s, c) — this keeps every device-side pass and the
# store DMA fully contiguous.
PERM = list(range(16)) + [21, 22, 16, 17, 18, 19, 20, 23, 24]


def _consts():
    W = _build_weights()
    idn = np.eye(128, dtype=np.float32)
    hsc = _hour_consts()
    # 24*eps' per ft channel; ft19 (rsi via u=1/(1+rs)) has scale -100
    epsAB = np.full(NFT, 24.0 * IN_EPS, dtype=np.float32)
    epsAB[19] = np.float32(24.0 * IN_EPS / 1e4)
    return {"w": W, "idn": idn, "hsc": hsc, "epsab": epsAB}


# --------------------------------------------------------------------------
# kernel body (Tile)
# --------------------------------------------------------------------------

def kernel_body(tc, outs, ins, repeat=1):
    import concourse.bass as bass
    from concourse import mybir

    nc = tc.nc
    f32 = mybir.dt.float32
    f32r = mybir.dt.float32r
    i32 = mybir.dt.int32
    AF = mybir.ActivationFunctionType
    OP = mybir.AluOpType
    AX = mybir.AxisListType

    x_d = ins["x"]            # [b_core, 120]
    w_d = ins["w"]            # [118, NMM]
    idn_d = ins["idn"]        # [128, 128]
    hsc_d = ins["hsc"]        # [48]
    epsab_d = ins["epsab"]    # [21]
    out_d = outs["out"]       # [b_core, 600]

    b_core = x_d.shape[0]
    assert b_core % BLOCK == 0
    nblocks = b_core // BLOCK

    from contextlib import ExitStack
    with ExitStack() as ctx:
        consts = ctx.enter_context(tc.tile_pool(name="consts", bufs=1))
        raw_p = ctx.enter_context(tc.tile_pool(name="raw", bufs=2))
        ft_p = ctx.enter_context(tc.tile_pool(name="ft", bufs=4))
        pk_p = ctx.enter_context(tc.tile_pool(name="pk", bufs=2))
        s2x_p = ctx.enter_context(tc.tile_pool(name="s2x", bufs=2))
        it_p = ctx.enter_context(tc.tile_pool(name="it", bufs=2))
        scr_p = ctx.enter_context(tc.tile_pool(name="scr", bufs=2))
        st_p = ctx.enter_context(tc.tile_pool(name="st", bufs=2))
        fsq_p = ctx.enter_context(tc.tile_pool(name="fsq", bufs=2))
        tps_p = ctx.enter_context(tc.tile_pool(name="tps", bufs=1, space="PSUM"))
        mps_p = ctx.enter_context(tc.tile_pool(name="mps", bufs=3, space="PSUM"))

        # ---- constants into SBUF ----
        mmdt = f32r if F32R else f32
        idn_t = consts.tile([128, 128], f32)
        nc.sync.dma_start(out=idn_t[:], in_=idn_d)
        w_raw = consts.tile([NPK, NMM], f32)
        nc.sync.dma_start(out=w_raw[:], in_=w_d)
        w_t = consts.tile([NPK, NMM], mmdt)
        nc.scalar.copy(out=w_t[:], in_=w_raw[:])

        def bcast_load(dst, src_ap):
            # DMA-broadcast a [n] dram vector to [128, n] sbuf
            src = bass.AP(tensor=src_ap.tensor, offset=src_ap.offset,
                          ap=[[0, 128]] + [list(p) for p in src_ap.ap])
            nc.sync.dma_start(out=dst, in_=src)

        hsc_t = consts.tile([128, 48], f32)
        bcast_load(hsc_t[:], hsc_d)
        epsab_t = consts.tile([128, NFT], f32)
        bcast_load(epsab_t[:], epsab_d)
        epsln_t = consts.tile([128, 1], f32)      # bias tile for Ln(close+EPS)
        nc.vector.memset(epsln_t[:], EPS)

        xr = x_d.rearrange("(blk p g) f -> blk p g f", p=PB, g=G)
        orr = out_d.rearrange("(blk p g) f -> blk p g f", p=PB, g=G)

        def phase1(blk):
            # ---------------- load ----------------
            raw = raw_p.tile([128, G, 120], f32)
            nc.sync.dma_start(out=raw[:], in_=xr[blk])
            rawv = raw.rearrange("p g (s c) -> p g s c", c=CIN)

            # ---------------- raw channels into c-major ft ----------------
            # ft channels: 0..15 -> out 0..15; 16=mom3, 17=mom6, 18=ch20,
            # 19=u (rsi), 20=obv, 21/22=hour, 23/24=zeros. ft IS the store
            # tile (channel-major DRAM layout; host permutes+transposes).
            ft = ft_p.tile([128, G, COUT, S], f32)
            nc.scalar.copy(
                out=ft[:, :, 0:3, :],
                in_=rawv[:, :, :, 0:3].rearrange("p g s c -> p g c s"))
            nc.gpsimd.tensor_copy(
                ft[:, :, 3:5, :],
                rawv[:, :, :, 3:5].rearrange("p g s c -> p g c s"))
            close = ft[:, :, 3, :]
            vol = ft[:, :, 4, :]
            # hour consts + zero channels (batch-independent, no stats)
            hql = hsc_t.rearrange("p (c s) -> p c s", c=2)
            nc.scalar.copy(
                out=ft[:, :, 21:23, :],
                in_=hql.unsqueeze(1).to_broadcast((128, G, 2, S)))
            nc.gpsimd.memset(ft[:, :, 23:25, :], 0.0)

            # ---------------- b-on-P pre-ops -> transposed pack -----------
            lc = scr_p.tile([128, G, S], f32, tag="lc")
            nc.scalar.activation(lc[:], close, AF.Ln, bias=epsln_t[:], scale=1.0)
            dl = scr_p.tile([128, G, 23], f32, tag="dl")
            nc.vector.tensor_tensor(dl[:], close[:, :, 1:24], close[:, :, 0:23],
                                    OP.subtract)
            pkT = pk_p.tile([128, G, NPK], f32)
            nc.scalar.copy(
                out=pkT[:, :, 0:48],
                in_=ft[:, :, 3:5, :].rearrange("p g c s -> p g (c s)"))
            nc.vector.tensor_scalar_max(pkT[:, :, 48:71], dl[:], 0.0)   # gain
            nc.gpsimd.tensor_tensor(pkT[:, :, 71:94], pkT[:, :, 48:71],
                                    dl[:], OP.subtract)                 # loss
            sg = scr_p.tile([128, G, 23], f32, tag="sg")
            nc.scalar.activation(sg[:], dl[:], AF.Sign)
            nc.gpsimd.tensor_tensor(pkT[:, :, 94:117], sg[:],
                                    vol[:, :, 1:24], OP.mult)           # vmult
            nc.vector.memset(pkT[:, :, 117:118], 1.0)

            # ---------------- in-transposes (PE) ----------------
            tpA = tps_p.tile([NPK, 512], f32, tag="tpA")
            tpB = tps_p.tile([NPK, 512], f32, tag="tpB")
            for g in range(4):
                nc.tensor.transpose(tpA[:, g * 128 : (g + 1) * 128],
                                    pkT[:, g, :], idn_t[:])
            for g in range(4):
                nc.tensor.transpose(tpB[:, g * 128 : (g + 1) * 128],
                                    pkT[:, 4 + g, :], idn_t[:])
            s2x = s2x_p.tile([NPK, G * 128], mmdt)
            nc.scalar.copy(out=s2x[:, 0:512], in_=tpA[:])
            nc.scalar.copy(out=s2x[:, 512:1024], in_=tpB[:])

            # ---------------- matmuls (waves of 2 chunks) ----------------
            it = it_p.tile([128, G, NMM], f32)
            for w in range(G // 2):
                mp = mps_p.tile([128, 2, 512], f32)
                for gg in range(2):
                    j = w * 2 + gg
                    nc.tensor.matmul(mp[:, gg, 0:NMM],
                                     s2x[:, j * 128 : (j + 1) * 128],
                                     w_t[:], start=True, stop=True)
                nc.scalar.copy(out=it[:, 2 * w : 2 * w + 2, :],
                               in_=mp[:, :, 0:NMM])
            itv = it.rearrange("p g (k s) -> p g k s", s=S)

            # ---------------- shifted-AP features (b-on-P, contiguous) ----
            nc.vector.tensor_tensor(ft[:, :, 5, 1:24], lc[:, :, 1:24],
                                    lc[:, :, 0:23], OP.subtract)        # ret
            nc.vector.memset(ft[:, :, 5, 0:1], 0.0)
            nc.vector.tensor_tensor(ft[:, :, 16, 3:24], lc[:, :, 3:24],
                                    lc[:, :, 0:21], OP.subtract)        # mom3
            nc.gpsimd.memset(ft[:, :, 16, 0:3], 0.0)
            nc.vector.tensor_tensor(ft[:, :, 17, 6:24], lc[:, :, 6:24],
                                    lc[:, :, 0:18], OP.subtract)        # mom6
            nc.gpsimd.memset(ft[:, :, 17, 0:6], 0.0)
            nc.scalar.activation(ft[:, :, 6, :], ft[:, :, 5, :], AF.Abs)

            # ---------------- nonlinear features ----------------
            # reciprocals via exp(-ln(x)) on ACT (~2 ULP)
            rc = scr_p.tile([128, G, S], f32, tag="rc")
            nc.scalar.activation(rc[:], lc[:], AF.Exp, scale=-1.0)
            rsv = scr_p.tile([128, G, 48], f32, tag="rsv")   # rS | rv
            nc.scalar.activation(rsv[:], it[:, :, 168:216], AF.Ln)
            nc.scalar.activation(rsv[:], rsv[:], AF.Exp, scale=-1.0)
            rS = rsv[:, :, 0:24]
            rv = rsv[:, :, 24:48]
            r12 = scr_p.tile([128, G, S], f32, tag="r12")
            nc.scalar.activation(r12[:], it[:, :, 48:72], AF.Ln)
            nc.scalar.activation(r12[:], r12[:], AF.Exp, scale=-1.0)

            # ema ratios ft9..12 = ema_k_s * rc
            rc4 = rc.unsqueeze(2).to_broadcast((128, G, 4, S))
            nc.gpsimd.tensor_tensor(ft[:, :, 9:13, :], itv[:, :, 0:4, :],
                                    rc4, OP.mult)
            # vol ratios ft13..14
            rv2 = rv.unsqueeze(2).to_broadcast((128, G, 2, S))
            nc.gpsimd.tensor_tensor(ft[:, :, 13:15, :], itv[:, :, 4:6, :],
                                    rv2, OP.mult)
            # ft7 = (high-low)*rc
            hl = scr_p.tile([128, G, S], f32, tag="hl")
            nc.vector.tensor_tensor(hl[:], ft[:, :, 1, :], ft[:, :, 2, :],
                                    OP.subtract)
            nc.gpsimd.tensor_tensor(ft[:, :, 7, :], hl[:], rc[:], OP.mult)
            # ft8 = |open-close|*rc
            oc = scr_p.tile([128, G, S], f32, tag="oc")
            nc.vector.tensor_tensor(oc[:], ft[:, :, 0, :], close, OP.subtract)
            nc.scalar.activation(oc[:], oc[:], AF.Abs)
            nc.gpsimd.tensor_tensor(ft[:, :, 8, :], oc[:], rc[:], OP.mult)
            # ft15 = log1p(vol)
            nc.scalar.activation(ft[:, :, 15, :], vol, AF.Ln, bias=1.0)
            # ft18 = close * r12  (the -1 shift is normalization-invariant)
            nc.vector.tensor_tensor(ft[:, :, 18, :], close, r12[:], OP.mult)
            # ft19 = u = (avgL+eps)*rS   (rsi = 100-100u; sign via -A)
            nc.vector.tensor_tensor(ft[:, :, 19, :], it[:, :, 144:168],
                                    rS, OP.mult)
            # ft20 = obv
            nc.scalar.copy(out=ft[:, :, 20, :], in_=it[:, :, 216:240])

            return dict(ft=ft)

        def phase2(blk, t):
            ft = t['ft']
            fts = ft[:, :, 0:NFT, :]        # stats channels
            # ---------------- stats: sums + sum of squares ----------------
            sums = st_p.tile([128, G, NFT], f32, tag="sums")
            nc.vector.reduce_sum(sums[:], fts, axis=AX.X)
            fsq = fsq_p.tile([128, G, NFT, S], f32)
            nc.scalar.activation(fsq[:], fts, AF.Square)
            sumsq = st_p.tile([128, G, NFT], f32, tag="sumsq")
            nc.vector.reduce_sum(sumsq[:], fsq[:], axis=AX.X)

            # negm = -sums/24; var*24 + 24*eps' = sumsq + negm*sums + 24*eps'
            negm = st_p.tile([128, G, NFT], f32, tag="negm")
            nc.vector.tensor_scalar_mul(negm[:], sums[:], -1.0 / 24.0)
            m2t = st_p.tile([128, G, NFT], f32, tag="m2t")
            nc.gpsimd.tensor_tensor(m2t[:], negm[:], sums[:], OP.mult)
            nc.gpsimd.tensor_tensor(m2t[:], sumsq[:], m2t[:], OP.add)
            nc.gpsimd.tensor_tensor(
                m2t[:], m2t[:],
                epsab_t[:].unsqueeze(1).to_broadcast((128, G, NFT)), OP.add)
            # inv = rsqrt(var + eps') = exp(-0.5*ln(m2t/24))
            inv = st_p.tile([128, G, NFT], f32, tag="inv")
            nc.scalar.activation(inv[:], m2t[:], AF.Ln, scale=1.0 / 24.0)
            nc.scalar.activation(inv[:], inv[:], AF.Exp, scale=-0.5)
            # rsi channel: out = -norm(u)
            nc.vector.tensor_scalar_mul(inv[:, :, 19:20], inv[:, :, 19:20], -1.0)
            bt = st_p.tile([128, G, NFT], f32, tag="bt")    # B = negm*A
            nc.gpsimd.tensor_tensor(bt[:], negm[:], inv[:], OP.mult)

            # -------- normalize: two contiguous in-place passes -----------
            invb = inv.unsqueeze(3).to_broadcast((128, G, NFT, S))
            btb = bt.unsqueeze(3).to_broadcast((128, G, NFT, S))
            nc.vector.tensor_tensor(ft[:, 0:5, 0:NFT], ft[:, 0:5, 0:NFT],
                                    invb[:, 0:5], OP.mult)
            nc.gpsimd.tensor_tensor(ft[:, 5:8, 0:NFT], ft[:, 5:8, 0:NFT],
                                    invb[:, 5:8], OP.mult)
            nc.vector.tensor_tensor(ft[:, 0:5, 0:NFT], ft[:, 0:5, 0:NFT],
                                    btb[:, 0:5], OP.add)
            nc.gpsimd.tensor_tensor(ft[:, 5:8, 0:NFT], ft[:, 5:8, 0:NFT],
                                    btb[:, 5:8], OP.add)

            # ---------------- store (channel-major DRAM layout) -----------
            nc.sync.dma_start(out=orr[blk],
                              in_=ft.rearrange("p g c s -> p g (c s)"))

        seq = [b for _ in range(repeat) for b in range(nblocks)]
        pend = []
        for blk in seq:
            t = phase1(blk)
            pend.append((blk, t))
            if len(pend) > 2:
                phase2(*pend.pop(0))
        for p in pend:
            phase2(*p)


# --------------------------------------------------------------------------
# host wrapper
# --------------------------------------------------------------------------

_CACHE = {}


def _compiled(b_core, repeat=1):
    if (b_core, repeat) in _CACHE:
        return _CACHE[(b_core, repeat)]
    import concourse.bacc as bacc
    import concourse.tile as tile
    from concourse import mybir, hw_specs

    f32 = mybir.dt.float32
    nc = bacc.Bacc("TRN2", target_bir_lowering=False, debug=False)
    ins = {
        "x": nc.dram_tensor("x", [b_core, 120], f32, kind="ExternalInput").ap(),
        "w": nc.dram_tensor("w", [NPK, NMM], f32, kind="ExternalInput").ap(),
        "idn": nc.dram_tensor("idn", [128, 128], f32, kind="ExternalInput").ap(),
        "hsc": nc.dram_tensor("hsc", [48], f32, kind="ExternalInput").ap(),
        "epsab": nc.dram_tensor("epsab", [NFT], f32, kind="ExternalInput").ap(),
    }
    outs = {
        "out": nc.dram_tensor("out", [b_core, 600], f32,
                              kind="ExternalOutput").ap(),
    }
    with tile.TileContext(nc) as tc:
        kernel_body(tc, outs, ins, repeat=repeat)

    # Pin every activation to one table set so the compiler emits a single
    # ACT_TABLE_LOAD instead of thrashing between per-function sets.
    tables = hw_specs.get_activation_tables(nc.m.arch)
    saved = {k: set(v) for k, v in tables.items()}
    try:
        for k in tables:
            if k != ACT_SET:
                tables[k] = set()
        nc.compile()
    finally:
        for k, v in saved.items():
            tables[k] = v
    _CACHE[(b_core, repeat)] = nc
    return nc


def kernel(x):
    from concourse import bass_utils

    x = np.ascontiguousarray(np.asarray(x, dtype=np.float32))
    B = x.shape[0]
    assert B % NCORES == 0
    b_core = B // NCORES
    consts = _consts()
    nc = _compiled(b_core)
    xf = x.reshape(B, S * CIN)
    in_maps = [
        {"x": np.ascontiguousarray(xf[i * b_core : (i + 1) * b_core]), **consts}
        for i in range(NCORES)
    ]
    res = bass_utils.run_bass_kernel_spmd(nc, in_maps, core_ids=list(range(NCORES)))
    out = np.concatenate([r["out"] for r in res.results], axis=0)
    # device emits channel-major [b, 25ft, 24s]; gather out-channel order
    # and flip to [b, s, c]
    out = out.reshape(B, COUT, S)[:, PERM, :].transpose(0, 2, 1)
    return np.ascontiguousarray(out)

